# revision 1
# baseline (speedup 1.0000x reference)
"""KMeansPalettizedConv2d on 8 TRN2 NeuronCores.

Strategy (data-parallel, per sharding hint):
- Shard the batch (32 images) 4-per-core across 8 cores. Replicate the
  compressed weights (int16 palette indices), the 256-entry lookup table,
  and the bias to every core.
- On device, each core dequantizes the weights with 32 GpSimd ap_gather
  instructions (256-entry f32 table lookup), redistributes the
  16-partition-replicated gather output into lhsT tiles with
  partition-strided SBUF->SBUF DMAs, casts to bf16, and runs the 3x3 conv
  as 18 accumulating bf16 matmuls (9 taps x 2 Cin chunks) per output tile
  in PSUM, reusing each weight tile across the 4 images. Bias is fused into
  the PSUM->SBUF drain on the scalar engine.
- Host work is limited to layout prep: zero-padding + bf16 cast of the
  input, index re-layout (int32 -> wrapped int16), and output reassembly.
"""
import numpy as np
import ml_dtypes

import concourse.mybir as mybir
import concourse.tile as tile
from concourse import bacc
from concourse.bass_utils import run_bass_kernel_spmd

BF16 = mybir.dt.bfloat16
F32 = mybir.dt.float32
I16 = mybir.dt.int16

N_CORES = 8
N_IMG = 4           # images per core
HP = 58             # padded spatial
NI = 2304           # idx per ap_gather instr: 18 tiles x 128 co
_NC_CACHE = {}


def build_nc(loop_reps=None):
    """Build the per-core Bass program. loop_reps wraps dequant+conv in a
    hardware For_i loop (for timing); None = straight-line (for grading)."""
    nc = bacc.Bacc("TRN2", target_bir_lowering=False, debug=False,
                   num_devices=N_CORES)
    xin = nc.dram_tensor("xin", [N_IMG, 2, 128, HP * HP], BF16, kind="ExternalInput")
    tab = nc.dram_tensor("tab", [128, 256], F32, kind="ExternalInput")
    widx = nc.dram_tensor("widx", [32, 128, NI // 16], I16, kind="ExternalInput")
    bias2 = nc.dram_tensor("bias2", [128, 2], F32, kind="ExternalInput")
    out = nc.dram_tensor("out", [N_IMG, 2, 128, 56 * 56], F32, kind="ExternalOutput")

    with tile.TileContext(nc) as tc:
        with (
            tc.tile_pool(name="tabp", bufs=1) as tabp,
            tc.tile_pool(name="idxp", bufs=1) as idxp,
            tc.tile_pool(name="biasp", bufs=1) as biasp,
            tc.tile_pool(name="imgp", bufs=2 * N_IMG) as imgp,
            tc.tile_pool(name="slab", bufs=3) as slabp,
            tc.tile_pool(name="wf", bufs=1) as wfp,
            tc.tile_pool(name="wb", bufs=1) as wbp,
            tc.tile_pool(name="ps", bufs=8, space="PSUM") as psp,
            tc.tile_pool(name="ob", bufs=8) as obp,
        ):
            tab_sb = tabp.tile([128, 256], F32)
            nc.sync.dma_start(tab_sb[:], tab[:])
            idx_sb = idxp.tile([128, 32 * (NI // 16)], I16)
            for i in range(32):
                nc.sync.dma_start(
                    idx_sb[:, i * (NI // 16):(i + 1) * (NI // 16)], widx[i])
            b_sb = biasp.tile([128, 2], F32)
            nc.sync.dma_start(b_sb[:], bias2[:])

            img_sb = [[None] * 2 for _ in range(N_IMG)]
            for i in range(N_IMG):
                for a in range(2):
                    t = imgp.tile([128, HP, HP], BF16)
                    nc.sync.dma_start(t[:], xin[i, a])
                    img_sb[i][a] = t

            wf32 = wfp.tile([128, 4608], F32)
            wbf = wbp.tile([128, 4608], BF16)

            def body():
                for b in range(2):
                    # --- dequant weights for cout-chunk b ---
                    for r in range(16):
                        s = slabp.tile([128, NI], F32)
                        ii = b * 16 + r
                        nc.gpsimd.ap_gather(
                            s[:], tab_sb[:],
                            idx_sb[:, ii * (NI // 16):(ii + 1) * (NI // 16)],
                            channels=128, num_elems=256, d=1, num_idxs=NI)
                        nc.sync.dma_start(
                            wf32[r::16, b * NI:(b + 1) * NI], s[r::16, :])
                    nc.vector.tensor_copy(
                        wbf[:, b * NI:(b + 1) * NI], wf32[:, b * NI:(b + 1) * NI])
                    # --- conv for cout-chunk b ---
                    for rt in range(7):
                        psum = [psp.tile([128, 448], F32, tag="psum", name="psum") for _ in range(N_IMG)]
                        for a in range(2):
                            for kk in range(9):
                                ky, kx = kk // 3, kk % 3
                                t_idx = b * 18 + a * 9 + kk
                                w_ap = wbf[:, t_idx * 128:(t_idx + 1) * 128]
                                first = (a == 0 and kk == 0)
                                last = (a == 1 and kk == 8)
                                for i in range(N_IMG):
                                    rhs = img_sb[i][a][
                                        :, rt * 8 + ky: rt * 8 + ky + 8, kx: kx + 56]
                                    nc.tensor.matmul(psum[i][:], w_ap, rhs,
                                                     start=first, stop=last)
                        for i in range(N_IMG):
                            o = obp.tile([128, 448], F32)
                            nc.vector.tensor_scalar_add(
                                o[:], psum[i][:], b_sb[:, b:b + 1])
                            nc.sync.dma_start(
                                out[i, b][:, rt * 448:(rt + 1) * 448], o[:])

            # straight-line repetition for timing builds (For_i + ap_gather
            # hard-faults the NC, so no hardware loop here)
            for _ in range(1 if loop_reps is None else loop_reps):
                body()
    nc.finalize()
    return nc


def prep_inputs(input, weight_idx, lookup_table, bias):
    input = np.asarray(input)
    weight_idx = np.asarray(weight_idx)
    lookup_table = np.asarray(lookup_table, dtype=np.float32)
    bias = np.asarray(bias, dtype=np.float32)

    xp = np.zeros((32, 256, HP, HP), np.float32)
    xp[:, :, 1:57, 1:57] = input
    xin = xp.reshape(32, 2, 128, HP * HP).astype(ml_dtypes.bfloat16)

    # wrapped gather indices: instr (b, r) covers, for Q7 group g, the weight
    # row of partition p = 16g + r (ci = a*128 + p), columns (a, kk, co).
    A = weight_idx.reshape(2, 128, 2, 8, 16, 9)          # [b, co, a, g, r, kk]
    L = A.transpose(0, 4, 3, 2, 5, 1).reshape(2, 16, 8, NI)  # [b, r, g, j]
    # wrapped storage: logical j at partition 16g + (j%16), column j//16
    widx = (L.reshape(2, 16, 8, NI // 16, 16)
             .transpose(0, 1, 2, 4, 3)
             .reshape(32, 128, NI // 16)
             .astype(np.int16))

    tab = np.broadcast_to(lookup_table, (128, 256)).copy()
    bias2 = np.ascontiguousarray(bias.reshape(2, 128).T)

    in_maps = []
    for c in range(N_CORES):
        in_maps.append({
            "xin": xin[c * N_IMG:(c + 1) * N_IMG],
            "tab": tab,
            "widx": widx,
            "bias2": bias2,
        })
    return in_maps


def run(in_maps, loop_reps=None, cores=None):
    key = loop_reps
    if key not in _NC_CACHE:
        _NC_CACHE[key] = build_nc(loop_reps)
    if cores is None:
        cores = list(range(N_CORES))
    res = run_bass_kernel_spmd(_NC_CACHE[key], in_maps[:len(cores)],
                               core_ids=cores)
    return res


def kernel(input, weight_idx, lookup_table, bias):
    in_maps = prep_inputs(input, weight_idx, lookup_table, bias)
    res = run(in_maps)
    outs = [res.results[c]["out"] for c in range(N_CORES)]
    full = np.concatenate(outs, axis=0)          # [32, 2, 128, 3136]
    return full.reshape(32, 256, 56, 56)

